# revision 11
# baseline (speedup 1.0000x reference)
"""GCN propagation kernel for Trainium2, 8 NeuronCores.

Computes out = D^-1/2 A D^-1/2 X W  with A [16384,16384] f32, X [16384,256] f32,
W [256,256] f32, D = diag(rowsum(A)).

Strategy (1D row partition, 8 cores):
  - Each core owns 2048 rows of A. Host pre-transposes + casts its shard to
    fp16: adjT_c = A[rows_c, :].T  -> [16384, 2048]  (contraction dim on
    partitions for the PE).
  - Pass 1: stream adjT (64 MiB), rowsum via PE (ones-stationary matmuls)
    -> local deg [2048].  feat (fp16, replicated) is prefetched meanwhile.
  - AllGather local deg -> full deg [16384] (tiny collective).
  - d = rsqrt(max(deg, eps)) on chip (DVE reciprocal + ACT sqrt), then
    x1 = feat * d[row] in fp16 (per-partition scalars).
  - Pass 2: stream adjT again, x2^T [256, 2048] accumulated in all 8 PSUM
    banks over the full contraction (x1 k-slices stationary, adjT moving).
  - Epilogue: x2T -> SBUF, GEMM2 with W (fp32), PE-transpose back to row
    orientation, scale by local d (the outer D^-1/2 commutes past W), DMA out.

fp16 is used only for the A-matmul inputs; all accumulation is fp32.
End-to-end error vs the fp32 reference is ~3e-4 (L2 relative).
"""

import numpy as np

import concourse.bass as bass
import concourse.tile as tile
from concourse import bacc, mybir
from concourse import bass_utils
from concourse.masks import make_identity

NCORES = 8
N = 16384          # nodes
D = 256            # feature dim (in == out)
M = N // NCORES    # 2048 local rows per core
P = 128            # partitions
KB = N // P        # 128 contraction blocks
MB = M // P        # 16 local row blocks
NCH = M // 512     # 4 moving-dim chunks of 512
EPS = 1e-12

F16 = mybir.dt.float16
F32 = mybir.dt.float32


def _build():
    nc = bacc.Bacc("TRN2", target_bir_lowering=False, debug=False, num_devices=NCORES)
    adjT = nc.dram_tensor("adjT", [N, M], F16, kind="ExternalInput").ap()
    featq = nc.dram_tensor("featq", [N, D], F16, kind="ExternalInput").ap()
    wgt = nc.dram_tensor("wgt", [D, D], F32, kind="ExternalInput").ap()
    out = nc.dram_tensor("out", [M, D], F32, kind="ExternalOutput").ap()

    with tile.TileContext(nc) as tc:
        with tc.tile_pool(name="const", bufs=1) as cpool, \
             tc.tile_pool(name="x1p", bufs=1) as x1p, \
             tc.tile_pool(name="adjp", bufs=8) as adjp, \
             tc.tile_pool(name="big", bufs=1) as big, \
             tc.tile_pool(name="small", bufs=1) as small, \
             tc.tile_pool(name="outp", bufs=4) as outp, \
             tc.tile_pool(name="ps", bufs=1, space="PSUM") as ps, \
             tc.tile_pool(name="dram", bufs=1, space="DRAM") as dram:

            ones = cpool.tile([P, 1], F16)
            nc.vector.memset(ones[:], 1.0)
            ident = cpool.tile([P, P], F32)
            make_identity(nc, ident[:])
            w_sb = cpool.tile([P, 2 * D], F32)  # w_sb[:, nb*D:(nb+1)*D] = W[nb*128:(nb+1)*128, :]
            nc.scalar.dma_start(
                w_sb[:].rearrange("p (nb o) -> p nb o", nb=2),
                wgt.rearrange("(nb p) o -> p nb o", p=P),
            )

            # x1 (= scaled feat) lives for the whole kernel: [128, KB*D] fp16
            x1_all = x1p.tile([P, KB * D], F16)

            # PSUM is managed as 8 shared bank slots (tags bank0..bank7);
            # tiles in different phases reuse banks as lifetimes allow.
            # deg accumulators: 4 PSUM banks of [1, 512]
            deg_ps = [ps.tile([P, 512], F32, name=f"deg_ps{i}", tag=f"bank{i}")[:1, :] for i in range(NCH)]

            # ---------------- pass 1: rowsum(adjT) + feat prefetch ----------------
            for kb in range(KB):
                adjt = adjp.tile([P, M], F16, name="adjt", tag="adjt")
                nc.sync.dma_start(adjt[:], adjT[kb * P:(kb + 1) * P, :])
                for mc in range(NCH):
                    nc.tensor.matmul(
                        deg_ps[mc][:, :],
                        ones[:, :],
                        adjt[:, mc * 512:(mc + 1) * 512],
                        start=(kb == 0),
                        stop=(kb == KB - 1),
                    )
                if kb % 16 == 0:
                    c0 = kb  # feat chunk: k-tiles [c0, c0+16)
                    nc.sync.dma_start(
                        x1_all[:, c0 * D:(c0 + 16) * D].rearrange("p (t f) -> p t f", t=16),
                        featq[c0 * P:(c0 + 16) * P, :].rearrange("(t p) f -> p t f", p=P),
                    )

            # ---------------- deg -> AllGather -> d ----------------
            deg_row = small.tile([1, M], F32)
            for mc in range(NCH):
                nc.vector.tensor_copy(deg_row[:, mc * 512:(mc + 1) * 512], deg_ps[mc][:, :])
            deg_loc_dram = dram.tile([M], F32)
            nc.scalar.dma_start(deg_loc_dram.rearrange("(p f) -> p f", p=1), deg_row[:, :])
            deg_all_dram = dram.tile([N], F32, addr_space="Shared")
            nc.gpsimd.collective_compute(
                "AllGather",
                mybir.AluOpType.bypass,
                replica_groups=[list(range(NCORES))],
                ins=[deg_loc_dram[:]],
                outs=[deg_all_dram[:]],
            )
            # natural layout [p, t] = deg[p*128 + t]
            d_nat = small.tile([P, P], F32)
            nc.scalar.dma_start(d_nat[:], deg_all_dram.rearrange("(p t) -> p t", p=P))
            nc.vector.tensor_scalar_max(d_nat[:], d_nat[:], EPS)
            d_rec = small.tile([P, P], F32)
            nc.vector.reciprocal(d_rec[:], d_nat[:])
            d_rs = small.tile([P, P], F32)
            nc.scalar.sqrt(d_rs[:], d_rec[:])
            # transpose so that d_all[p, t] = rsqrt(deg[t*128 + p])
            d_ps = ps.tile([P, 512], F32, name="d_ps", tag="bank4")[:, :P]
            nc.tensor.transpose(d_ps[:], d_rs[:], ident[:])
            d_all = small.tile([P, P], F32)
            nc.vector.tensor_copy(d_all[:], d_ps[:])

            # x1 = featq * d (in place, fp16)
            for t in range(KB):
                sl = x1_all[:, t * D:(t + 1) * D]
                nc.vector.tensor_scalar_mul(sl, sl, d_all[:, t:t + 1])

            # local d for the epilogue: d_loc[p, mb] = rsqrt(deg_local[mb*128+p])
            degl = small.tile([P, MB], F32)
            nc.scalar.dma_start(degl[:], deg_loc_dram.rearrange("(t p) -> p t", p=P))
            nc.vector.tensor_scalar_max(degl[:], degl[:], EPS)
            degl_rec = small.tile([P, MB], F32)
            nc.vector.reciprocal(degl_rec[:], degl[:])
            d_loc = small.tile([P, MB], F32)
            nc.scalar.sqrt(d_loc[:], degl_rec[:])

            # ---------------- pass 2: x2T[n, m] += x1[k, n]^T adjT[k, m] ----------------
            x2_ps = [[ps.tile([P, 512], F32, name=f"x2_ps{nb}_{mc}", tag=f"bank{nb * NCH + mc}")
                      for mc in range(NCH)] for nb in range(2)]
            for kb in range(KB):
                adjt = adjp.tile([P, M], F16, name="adjt", tag="adjt")
                nc.sync.dma_start(adjt[:], adjT[kb * P:(kb + 1) * P, :])
                for nb in range(2):
                    lhsT = x1_all[:, kb * D + nb * P: kb * D + (nb + 1) * P]
                    for mc in range(NCH):
                        nc.tensor.matmul(
                            x2_ps[nb][mc][:, :],
                            lhsT,
                            adjt[:, mc * 512:(mc + 1) * 512],
                            start=(kb == 0),
                            stop=(kb == KB - 1),
                        )

            # ---------------- epilogue ----------------
            x2_sb = [big.tile([P, M], F32, name=f"x2_sb{nb}", tag=f"x2_sb{nb}") for nb in range(2)]
            for nb in range(2):
                for mc in range(NCH):
                    nc.vector.tensor_copy(
                        x2_sb[nb][:, mc * 512:(mc + 1) * 512],
                        x2_ps[nb][mc][:, :],
                    )
            # GEMM2 (fp32): preT[o, m] = sum_n W[n, o] * x2T[n, m]
            pre_ps = [[ps.tile([P, 512], F32, name=f"pre_ps{ob}_{mc}", tag=f"bank{ob * NCH + mc}")
                       for mc in range(NCH)] for ob in range(2)]
            for ob in range(2):
                for mc in range(NCH):
                    for nb in range(2):
                        nc.tensor.matmul(
                            pre_ps[ob][mc][:, :],
                            w_sb[:, nb * D + ob * P: nb * D + (ob + 1) * P],
                            x2_sb[nb][:, mc * 512:(mc + 1) * 512],
                            start=(nb == 0),
                            stop=(nb == 1),
                        )
            pre_sb = [big.tile([P, M], F32, name=f"pre_sb{ob}", tag=f"pre_sb{ob}") for ob in range(2)]
            for ob in range(2):
                for mc in range(NCH):
                    nc.vector.tensor_copy(
                        pre_sb[ob][:, mc * 512:(mc + 1) * 512],
                        pre_ps[ob][mc][:, :],
                    )
            # transpose back to [m, o] blocks, scale by local d, store
            for mb in range(MB):
                out_t = outp.tile([P, D], F32, name="out_t", tag="out_t")
                for ob in range(2):
                    tp = ps.tile([P, 512], F32, name="tp", tag=f"bank{(mb * 2 + ob) % 8}")[:, :P]
                    nc.tensor.transpose(tp[:], pre_sb[ob][:, mb * P:(mb + 1) * P], ident[:])
                    nc.vector.tensor_scalar_mul(out_t[:, ob * P:(ob + 1) * P], tp[:], d_loc[:, mb:mb + 1])
                nc.sync.dma_start(out[mb * P:(mb + 1) * P, :], out_t[:])

    nc.compile()
    return nc


_NC_CACHE = []


def _get_nc():
    if not _NC_CACHE:
        _NC_CACHE.append(_build())
    return _NC_CACHE[0]


def kernel(adj, feat, weight):
    adj = np.asarray(adj, dtype=np.float32)
    feat = np.asarray(feat, dtype=np.float32)
    weight = np.asarray(weight, dtype=np.float32)
    assert adj.shape == (N, N) and feat.shape == (N, D) and weight.shape == (D, D)

    featq = feat.astype(np.float16)
    adj16 = adj.astype(np.float16)
    in_maps = []
    for c in range(NCORES):
        adjT_c = np.ascontiguousarray(adj16[c * M:(c + 1) * M, :].T)
        in_maps.append({"adjT": adjT_c, "featq": featq, "wgt": weight})

    nc = _get_nc()
    res = bass_utils.run_bass_kernel_spmd(nc, in_maps, core_ids=list(range(NCORES)))
    return np.concatenate([res.results[c]["out"] for c in range(NCORES)], axis=0)
